# revision 1
# baseline (speedup 1.0000x reference)
"""Trainium2 Bass kernel for nn_BondMatrixMessage (GNN bond-matrix message passing).

Per batch b (one NeuronCore each, B=8 => 8 cores):
    bw[e,(i,j)] = sum_k bond[e,k] * W[k,(i,j)]          (PE, bf16)
    m[e,i]      = sum_j bw[e,(i,j)] * atom[src[e],j]    (ACT/DVE mult + PE selector-reduce)
    out[t,:]    = sum_{e: tgt[e]=t} m[e,:]              (sorted-edge scatter-add)

Feature-major chunked layout: per 1024-edge tile, 8 chunks of 128 partitions,
chunk c partition p <-> (i = 4c + p%4, j = p//4).
  - srcg[p, e] = atom[src[e], p//4] is host-prepared (pure index/layout prep,
    same class as the host edge sort) and DMA-streamed per tile -- no SWDGE
    gathers on device.
  - bw_c = W2_c^T @ bondT_tile (PSUM fp32, F=1024, split in 512-col matmuls);
    chunks with route 'A' are ACT-evacuated to SBUF bf16 then DVE-multiplied
    at 2x; route 'D' chunks are DVE-multiplied straight from PSUM (1x).
    Routes alternate so PSUM WAR unlocks pace evenly across ACT/DVE.
  - mt (32, 1024) = sum_c S_c^T @ pt_c (accumulating PSUM); S_c[p,m] = [4c+p%4==m].
  - mt -> bf16 SBUF (ACT) -> DMA-transpose to m_all[p, s, i] = m[i, s*128+p]
    (hw-verified layout == the scatter's token-wrap order).
  - Scatter: edges host-sorted by target; processing order = 16 blocks of 1024
    edges by (sorted_pos % 16); same-target edges are consecutive in sorted
    order so each block has unique targets (max in-degree <= 16; dma_scatter_add
    races on duplicate indices WITHIN a call). All blocks scatter-add bf16 into
    one host-pre-zeroed DRAM accumulator (Tile serializes the WAW chain, so
    cross-call duplicates are safe); rows are 128-wide bf16 (256B stride, a
    scatter-add constraint) with a 32-wide payload.
  - Final: load acc[:, 0:32] p-major, convert bf16->f32, store out (4096, 32).
"""
import sys

sys.path.insert(0, "/opt/trn_rl_repo")

import numpy as np

from concourse import bacc, bass, mybir, tile, bass_utils

# problem constants (hardcoded per spec)
B = 8
N = 4096
E = 16384
D = 32          # atom dim
KB = 64         # bond dim
TIL = 1024      # edges per pipeline tile (= scatter block)
NT = E // TIL   # 16 tiles
CH = 8          # (j,i) chunks per tile
NBLK = 16       # sorted-mod blocks (requires max in-degree <= NBLK)
TPB = E // NBLK  # tokens per block = 1024
F32 = mybir.dt.float32
BF16 = mybir.dt.bfloat16
I16 = mybir.dt.int16

_PROGRAM_CACHE = {}

# tunables
CFG = dict(
    route="DADADAAA",  # per-chunk multiply route: D=DVE-direct-from-PSUM,
                       # A=ACT-evac+DVE-2x, P=ACT-evac+Pool-mult
    bw_bufs=3,       # PSUM bufs for bw chunk tiles (2 banks each)
    mt_bufs=1,       # PSUM bufs for the mT accumulator (2 banks each)
    pt_bufs=6,       # SBUF bufs for pt (multiplied products)
    bwsb_bufs=3,     # SBUF bufs for ACT-evacuated bw
    bt_bufs=3,       # SBUF bufs for bondT tiles
    sg_bufs=3,       # SBUF bufs for srcg tiles
    mt_sb_bufs=2,    # SBUF bufs for evacuated bf16 mt
    pipe_ahead=2,    # emit create matmul for chunk c+pipe_ahead before sel of c
    tail_at=1,       # chunk index of tile t after which tile t-1's tail is emitted
    warmup=8,        # PE warm-up matmuls (p-state ramp) while DMAs load
)


def _build_program(cfg=None):
    cfg = {**CFG, **(cfg or {})}
    nc = bacc.Bacc("TRN2", target_bir_lowering=False, debug=False, num_devices=B)

    bondT_d = nc.dram_tensor("bondT", (KB, E), BF16, kind="ExternalInput")
    srcg_d = nc.dram_tensor("srcg", (128, E), BF16, kind="ExternalInput")
    w2_d = nc.dram_tensor("w2", (KB, CH * 128), BF16, kind="ExternalInput")
    sel_d = nc.dram_tensor("sel", (128, CH * D), BF16, kind="ExternalInput")
    tgtw_d = nc.dram_tensor("tgtw", (128, E // 16), I16, kind="ExternalInput")
    acc_d = nc.dram_tensor("acc", (N, 128), BF16, kind="ExternalInput")  # pre-zeroed
    out_d = nc.dram_tensor("out", (N, D), F32, kind="ExternalOutput")

    route = cfg["route"]
    assert len(route) == CH and set(route) <= {"A", "D", "P"}

    with tile.TileContext(nc) as tc:
        with tc.tile_pool(name="const", bufs=1) as cp, \
             tc.tile_pool(name="ptp", bufs=cfg["pt_bufs"]) as wp, \
             tc.tile_pool(name="bwsb", bufs=cfg["bwsb_bufs"]) as bp, \
             tc.tile_pool(name="btp", bufs=cfg["bt_bufs"]) as btp, \
             tc.tile_pool(name="sgp", bufs=cfg["sg_bufs"]) as sgp, \
             tc.tile_pool(name="mtev", bufs=cfg["mt_sb_bufs"]) as mp, \
             tc.tile_pool(name="finp", bufs=2) as fp, \
             tc.tile_pool(name="bwps", bufs=cfg["bw_bufs"], space="PSUM") as bwp, \
             tc.tile_pool(name="mtps", bufs=cfg["mt_bufs"], space="PSUM") as mtp:

            # ---- PE warm-up: garbage matmuls with no input deps keep the PE
            # p-state ramp going while the first DMAs load (never read back) ----
            if cfg["warmup"]:
                wm_in = cp.tile([KB, 512], BF16, name="wm_in")
                nc.gpsimd.memset(wm_in[:], 0.0)
                wm_ps = mtp.tile([128, 512], F32, tag="mt", name="wm_ps")
                for w in range(cfg["warmup"]):
                    nc.tensor.matmul(
                        out=wm_ps[:], lhsT=wm_in[:, 0:128], rhs=wm_in[:],
                        start=True, stop=True, skip_group_check=True,
                    )

            # ---- setup; first-create dependencies first: chunk 0 of w2,
            # then tile-0 bond, then the rest ----
            w2_sb = cp.tile([KB, CH * 128], BF16)
            nc.sync.dma_start(w2_sb[:, 0:128], w2_d.ap()[:, 0:128])

            st = {}
            nq = TIL // 128

            def emit_loads(t):
                esl = slice(t * TIL, (t + 1) * TIL)
                bt_sb = btp.tile([KB, TIL], BF16, tag="bt", name="bt_sb")
                nc.sync.dma_start(bt_sb[:], bondT_d.ap()[:, esl])
                sg_sb = sgp.tile([128, TIL], BF16, tag="sg", name="sg_sb")
                nc.sync.dma_start(sg_sb[:], srcg_d.ap()[:, esl])
                st[t] = dict(bt=bt_sb, sg=sg_sb, bw={}, pt={}, mt=None)

            emit_loads(0)

            nc.sync.dma_start(w2_sb[:, 128:], w2_d.ap()[:, 128:])
            sel_sb = cp.tile([128, CH * D], BF16)
            nc.sync.dma_start(sel_sb[:], sel_d.ap())
            tgtw_sb = cp.tile([128, E // 16], I16)
            nc.scalar.dma_start(tgtw_sb[:], tgtw_d.ap())

            # edge-major bf16 messages, token-wrapped: token q at [q%128, q//128, :]
            m_all = cp.tile([128, E // 128, D], BF16)

            ahead = max(cfg["pipe_ahead"], 0)

            def emit_create(t, c):
                s = st[t]
                if s["mt"] is None:
                    s["mt"] = mtp.tile([D, TIL], F32, tag="mt", name="mt_ps")
                bw_ps = bwp.tile([128, TIL], F32, tag="bw", name="bw_ps")
                for h in range(TIL // 512):
                    hs = slice(h * 512, (h + 1) * 512)
                    nc.tensor.matmul(
                        out=bw_ps[:, hs],
                        lhsT=w2_sb[:, c * 128:(c + 1) * 128],
                        rhs=s["bt"][:, hs],
                        start=True, stop=True,
                    )
                s["bw"][c] = bw_ps

            def emit_mult(t, c):
                s = st[t]
                bw_ps = s["bw"].pop(c)
                pt_sb = wp.tile([128, TIL], BF16, tag="pt", name="pt_sb")
                if route[c] in "AP":
                    bw_sb = bp.tile([128, TIL], BF16, tag="bwsb", name="bw_sb")
                    nc.scalar.copy(bw_sb[:], bw_ps[:])
                    eng = nc.gpsimd if route[c] == "P" else nc.vector
                    eng.tensor_tensor(
                        out=pt_sb[:], in0=bw_sb[:], in1=s["sg"][:],
                        op=mybir.AluOpType.mult,
                    )
                else:
                    nc.vector.tensor_tensor(
                        out=pt_sb[:], in0=bw_ps[:], in1=s["sg"][:],
                        op=mybir.AluOpType.mult,
                    )
                s["pt"][c] = pt_sb

            def emit_sel(t, c):
                s = st[t]
                pt_sb = s["pt"].pop(c)
                for h in range(TIL // 512):
                    hs = slice(h * 512, (h + 1) * 512)
                    nc.tensor.matmul(
                        out=s["mt"][:, hs],
                        lhsT=sel_sb[:, c * D:(c + 1) * D],
                        rhs=pt_sb[:, hs],
                        start=(c == 0), stop=(c == CH - 1),
                    )

            def emit_tail(t, last=False):
                s = st[t]
                mt_sb = mp.tile([D, TIL], BF16, tag="mtsb", name="mt_sb")
                if last:
                    # halve the exposed evac latency: ACT and DVE in parallel
                    nc.scalar.copy(mt_sb[:, 0:TIL // 2], s["mt"][:, 0:TIL // 2])
                    nc.vector.tensor_copy(mt_sb[:, TIL // 2:], s["mt"][:, TIL // 2:])
                else:
                    nc.scalar.copy(mt_sb[:], s["mt"][:])
                sl0 = t * nq
                # hw-verified: writes m_all[p, sl0+s, i] = mt_sb[i, s*128+p]
                nc.sync.dma_start_transpose(m_all[:, sl0:sl0 + nq, :], mt_sb[:])
                nc.gpsimd.dma_scatter_add(
                    out_ap=acc_d.ap()[:, 0:D],
                    in_ap=m_all[:, sl0:sl0 + nq, :],
                    idxs_ap=tgtw_sb[:, t * (TPB // 16):(t + 1) * (TPB // 16)],
                    num_idxs=TPB,
                    num_idxs_reg=TPB,
                    elem_size=D,
                    elem_step=128,
                )
                del st[t]

            for t in range(NT):
                if t + 1 < NT:
                    emit_loads(t + 1)
                for c in range(CH):
                    emit_create(t, c)
                    emit_mult(t, c)
                    if c == cfg["tail_at"] and t >= 1:
                        emit_tail(t - 1)
                    if c >= ahead:
                        emit_sel(t, c - ahead)
                for r in range(min(ahead, CH)):
                    emit_sel(t, CH - min(ahead, CH) + r)
            emit_tail(NT - 1, last=True)

            # ---- final: acc[:, 0:32] (p-major rows n = 32p+g) -> f32 out,
            # split in halves so load/convert/store pipeline ----
            gh = (N // 128) // 2
            for v in range(2):
                gsl = slice(v * gh, (v + 1) * gh)
                acc_sb = fp.tile([128, gh, D], BF16, name="acc_sb")
                nc.sync.dma_start(
                    acc_sb[:],
                    acc_d.ap().rearrange("(p g) j -> p g j", p=128)[:, gsl, 0:D],
                )
                out_sb = fp.tile([128, gh * D], F32, name="out_sb")
                nc.vector.tensor_copy(
                    out_sb[:], acc_sb[:].rearrange("p g j -> p (g j)"))
                nc.sync.dma_start(
                    out_d.ap().rearrange("(p g) j -> p g j", p=128)[:, gsl, :]
                    .rearrange("p g j -> p (g j)"),
                    out_sb[:],
                )

    nc.compile()
    return nc


def _host_prep(atom_state, bond_state, bond_transform, connectivity):
    """Build per-core input maps. Pure layout / index-metadata / dtype prep."""
    import ml_dtypes

    W = np.asarray(bond_transform, dtype=np.float32)  # (KB, D*D)

    # W2[k, c*128 + p] = W[k, (4c + p%4)*D + p//4]   (i = 4c + p%4, j = p//4)
    p = np.arange(128)
    cc = np.arange(CH)
    i_idx = 4 * cc[:, None] + (p % 4)[None, :]   # (CH, 128)
    j_idx = np.broadcast_to((p // 4)[None, :], (CH, 128))
    w2 = W[:, (i_idx * D + j_idx).reshape(-1)].astype(ml_dtypes.bfloat16)

    # selectors S_c[p, m] = [4c + p%4 == m]
    sel = np.zeros((128, CH * D), dtype=np.float32)
    for c in range(CH):
        sel[p, c * D + 4 * c + (p % 4)] = 1.0
    sel_bf = sel.astype(ml_dtypes.bfloat16)

    zeros_acc = np.zeros((N, 128), dtype=ml_dtypes.bfloat16)

    in_maps = []
    for b in range(B):
        src = np.asarray(connectivity[b, :, 0], dtype=np.int64)
        tgt = np.asarray(connectivity[b, :, 1], dtype=np.int64)
        order = np.argsort(tgt, kind="stable")
        deg = np.bincount(tgt, minlength=N).max()
        if deg > NBLK:
            raise ValueError(f"max in-degree {deg} exceeds {NBLK}")
        # processing order: blocks by sorted_pos % NBLK
        proc = np.concatenate([order[c::NBLK] for c in range(NBLK)])
        tgtp = tgt[proc].astype(np.int16)

        bondT = np.ascontiguousarray(
            np.asarray(bond_state[b], dtype=np.float32).T[:, proc]
        ).astype(ml_dtypes.bfloat16)  # (KB, E)

        # srcg[p, e] = atom[src[proc[e]], p//4]
        atomg = np.asarray(atom_state[b], dtype=np.float32)[src[proc]]  # (E, D)
        srcg = np.ascontiguousarray(
            np.repeat(atomg.T.astype(ml_dtypes.bfloat16), 4, axis=0)
        )  # (128, E)

        # wrapped idx table: idxs[p, s] = vals[16*s + p%16], tiled to 128 partitions
        def wrap16(vals):
            w = vals.reshape(-1, 16).T  # (16, E//16)
            return np.ascontiguousarray(np.tile(w, (8, 1)), dtype=np.int16)

        in_maps.append({
            "bondT": bondT,
            "srcg": srcg,
            "w2": w2,
            "sel": sel_bf,
            "tgtw": wrap16(tgtp),
            "acc": zeros_acc,
        })
    return in_maps


def kernel(atom_state, bond_state, bond_transform, connectivity):
    if "nc" not in _PROGRAM_CACHE:
        _PROGRAM_CACHE["nc"] = _build_program()
    nc = _PROGRAM_CACHE["nc"]

    in_maps = _host_prep(atom_state, bond_state, bond_transform, connectivity)
    res = bass_utils.run_bass_kernel_spmd(nc, in_maps, list(range(B)))
    out = np.stack([res.results[b]["out"] for b in range(B)], axis=0)
    return out.astype(np.float32)

